# revision 9
# baseline (speedup 1.0000x reference)
"""CMoE hash-routed expert FFN on 8 NeuronCores (expert-parallel).

Host side (the shard/unshard steps): compute hash routing
e = (token_id % 5099) % 64, first-come slot assignment with capacity 512,
scatter tokens into a per-expert [E, D, C] buffer (transposed, bf16), and
shard 8 experts to each of the 8 cores along with that core's (transposed,
bf16) expert weights.  Device side: per expert
    h  = relu(A @ Wk^T)^2        [C, F]
    kv = h @ Wv^T                [C, D]
    r  = sigmoid(A @ Wr^T)       [C, D]
    out = r * kv
computed entirely in transposed form (contraction dim on SBUF partitions),
bf16 matmul operands with fp32 PSUM accumulation.  Host gathers each
token's slot back out of [E, D, C] and zeroes dropped tokens.

DMA discipline: transfers are issued in strict first-need order, chunked so
compute gates on partial tensors (wk in 4 f-chunks, wv in 2 fo-chunks), and
spread over three rings (sync HWDGE: at+wv, scalar HWDGE: wk, gpsimd SWDGE:
wr + output stores).  The r phase of expert e+1 is emitted after kv(e); its
at/wr loads are issued two experts ahead.  Outputs are stored as bf16 to
halve store traffic; the host upcasts.
"""

import numpy as np
import ml_dtypes

import concourse.bass as bass
import concourse.mybir as mybir
import concourse.tile as tile
from concourse import bacc
from concourse.bass import ts
from concourse.bass_utils import run_bass_kernel_spmd

HASH_PRIME = 5099
B, T, D, F, E = 8, 4096, 512, 1792, 64
S = B * T
C = 512  # capacity = max(4, ceil(S/E))
N_CORES = 8
E_LOC = E // N_CORES  # experts per core

BF16 = mybir.dt.bfloat16
F32 = mybir.dt.float32

_NC = None  # cached compiled Bass program
LAST_RESULT = None  # BassKernelResults of the most recent run (for test.py)


def _build_nc(e_loc=E_LOC, d=D, f=F, c=C):
    """One SPMD program: each core computes e_loc experts' FFN."""
    kd = d // 128   # contraction tiles over D
    kf = f // 128   # contraction tiles over F
    nc = bacc.Bacc("TRN2", target_bir_lowering=False, debug=False,
                   num_devices=N_CORES)

    # at and wr are packed host-side into one [2, d, c] block per expert so
    # the head-critical load (r needs both) is a single full-rate transfer
    awr_t = nc.dram_tensor("awr_t", [e_loc, 2, d, c], BF16, kind="ExternalInput")
    wk_t = nc.dram_tensor("wk_t", [e_loc, d, f], BF16, kind="ExternalInput")
    wv_t = nc.dram_tensor("wv_t", [e_loc, f, d], BF16, kind="ExternalInput")
    out_t = nc.dram_tensor("out_t", [e_loc, d, c], BF16, kind="ExternalOutput")

    with tile.TileContext(nc) as tc:
        with (
            tc.tile_pool(name="wts", bufs=2) as wts,
            tc.tile_pool(name="acts", bufs=2) as acts,
            tc.tile_pool(name="ph", bufs=3, space="PSUM") as ph,
            tc.tile_pool(name="pr", bufs=3, space="PSUM") as pr,
            tc.tile_pool(name="pkv", bufs=2, space="PSUM") as pkv,
        ):
            tiles_awr = {}
            tiles_wk = {}
            tiles_wv = {}
            sigs = {}
            hbs = {}

            # Warm the PE (HAM throttles it to 1.2 GHz until ~3.4us of
            # sustained work) with matmuls on scratch data while the first
            # input DMAs ramp up; the result is never read.  10 cold MMs
            # bridge ~4.3us, landing right when awr0 arrives.
            warm_l = wts.tile([128, 128], BF16, tag="warm_l")
            warm_r = wts.tile([128, c], BF16, tag="warm_r")
            nc.any.memset(warm_l[:], 0.0)
            nc.any.memset(warm_r[:], 0.0)
            for _ in range(10):
                warm_p = pr.tile([128, c], F32, tag="psr")
                nc.tensor.matmul(warm_p[:], lhsT=warm_l[:], rhs=warm_r[:],
                                 start=True, stop=True)

            def load_at_wr(e):
                # one 1MB transfer; SDMA round-robins across queues, so a
                # single transfer on its own ring completes ~2x faster than
                # two competing halves on separate rings
                awr = wts.tile([128, 2 * kd, c], BF16, tag="awr")
                tiles_awr[e] = awr
                nc.sync.dma_start(
                    awr[:],
                    awr_t[e].rearrange("two (ko p) c -> p (two ko) c", p=128))

            def load_wk(e):
                wk = wts.tile([128, kd, f], BF16, tag="wk")
                tiles_wk[e] = wk
                src = wk_t[e].rearrange("(ko p) f -> p ko f", p=128)
                # 4 chunks: h ft-groups gate on the 448-col chunk they read
                for lo in range(0, f, 512):
                    hi = min(lo + 512, f)
                    nc.scalar.dma_start(wk[:, :, lo:hi], src[:, :, lo:hi])

            def load_wv(e):
                wv = wts.tile([128, kf, d], BF16, tag="wv")
                tiles_wv[e] = wv
                src = wv_t[e].rearrange("(fo p) g -> p fo g", p=128)
                half = kf // 2
                nc.sync.dma_start(wv[:, :half, :], src[:, :half, :])
                nc.sync.dma_start(wv[:, half:, :], src[:, half:, :])

            def emit_r(e):
                awr = tiles_awr[e]  # [:, :kd] = A^T rows, [:, kd:] = Wr^T rows
                sig = acts.tile([128, kd, c], F32, tag="sig")
                sigs[e] = sig
                for gt in range(kd):
                    psum_r = pr.tile([128, c], F32, tag="psr")
                    for kt in range(kd):
                        nc.tensor.matmul(
                            psum_r[:],
                            lhsT=awr[:, kd + kt, ts(gt, 128)],
                            rhs=awr[:, kt, :],
                            start=(kt == 0),
                            stop=(kt == kd - 1),
                        )
                    nc.scalar.activation(sig[:, gt, :], psum_r[:],
                                         mybir.ActivationFunctionType.Sigmoid)

            def emit_h(e):
                awr = tiles_awr[e]
                wk = tiles_wk.pop(e)
                # h^T[f, c] = (relu(Wk^T.T @ A^T))^2, bf16 for matmul 2
                hb = acts.tile([128, kf, c], BF16, tag="hb")
                hbs[e] = hb
                for ft in range(kf):
                    psum_h = ph.tile([128, c], F32, tag="psh")
                    for kt in range(kd):
                        nc.tensor.matmul(
                            psum_h[:],
                            lhsT=wk[:, kt, ts(ft, 128)],
                            rhs=awr[:, kt, :],
                            start=(kt == 0),
                            stop=(kt == kd - 1),
                        )
                    nc.scalar.activation(hb[:, ft, :], psum_h[:],
                                         mybir.ActivationFunctionType.Relu)
                    nc.vector.tensor_mul(hb[:, ft, :], hb[:, ft, :], hb[:, ft, :])

            def emit_kv(e):
                tiles_awr.pop(e)
                wv = tiles_wv.pop(e)
                hb = hbs.pop(e)
                sig = sigs.pop(e)
                # kv^T[dd, c] = Wv^T.T @ h^T ; out = sig * kv
                ob = acts.tile([128, kd, c], BF16, tag="ob")
                for dt in range(kd):
                    psum_kv = pkv.tile([128, c], F32, tag="pskv")
                    for ft in range(kf):
                        nc.tensor.matmul(
                            psum_kv[:],
                            lhsT=wv[:, ft, ts(dt, 128)],
                            rhs=hb[:, ft, :],
                            start=(ft == 0),
                            stop=(ft == kf - 1),
                        )
                    nc.vector.tensor_mul(ob[:, dt, :], psum_kv[:], sig[:, dt, :])
                    # store each d-tile as it finishes; the last expert's
                    # stores ride the by-then-idle sync HWDGE ring (lower
                    # latency than SWDGE) to shorten the kernel tail
                    dst = out_t[e].rearrange("(ko p) c -> p ko c", p=128)[:, dt, :]
                    if e == e_loc - 1:
                        nc.sync.dma_start(dst, ob[:, dt, :])
                    else:
                        nc.gpsimd.dma_start(dst, ob[:, dt, :])

            # DMAs are issued in strict first-need order; compute for
            # expert e is emitted as r(e) | h(e) kv(e), with r one expert
            # ahead of h/kv so the PE always has work while wk/wv stream.
            load_at_wr(0)
            load_wk(0)
            emit_r(0)
            load_wv(0)
            if e_loc > 1:
                load_at_wr(1)
                load_wk(1)
            for e in range(e_loc):
                emit_h(e)
                emit_kv(e)
                if e + 1 < e_loc:
                    emit_r(e + 1)
                    load_wv(e + 1)
                if e + 2 < e_loc:
                    load_at_wr(e + 2)
                    load_wk(e + 2)

    nc.compile()
    return nc


def _route(token_ids):
    tid = token_ids.reshape(S).astype(np.int64)
    e_idx = (tid % HASH_PRIME) % E
    order = np.argsort(e_idx, kind="stable")
    sorted_e = e_idx[order]
    starts = np.searchsorted(sorted_e, np.arange(E))
    pos = np.empty(S, np.int64)
    pos[order] = np.arange(S) - starts[sorted_e]
    kept = pos < C
    return e_idx, pos, kept


def kernel(x, token_ids, Wk, Wr, Wv):
    global _NC, LAST_RESULT
    if _NC is None:
        _NC = _build_nc()

    e_idx, pos, kept = _route(token_ids)

    bf16 = ml_dtypes.bfloat16
    xf = np.ascontiguousarray(x, dtype=np.float32).reshape(S, D)
    # awr_t[e] = [A^T (dispatched) ; Wr^T] packed so the device loads both
    # with one transfer
    awr_t = np.zeros((E, 2, D, C), bf16)
    disp_t = np.zeros((E, D, C), np.float32)
    disp_t[e_idx[kept], :, pos[kept]] = xf[kept]
    awr_t[:, 0] = disp_t.astype(bf16)
    awr_t[:, 1] = np.asarray(Wr, dtype=np.float32).transpose(0, 2, 1).astype(bf16)

    wk_t = np.asarray(Wk, dtype=np.float32).transpose(0, 2, 1).astype(bf16)
    wv_t = np.asarray(Wv, dtype=np.float32).transpose(0, 2, 1).astype(bf16)

    in_maps = [
        {
            "awr_t": awr_t[i * E_LOC:(i + 1) * E_LOC],
            "wk_t": wk_t[i * E_LOC:(i + 1) * E_LOC],
            "wv_t": wv_t[i * E_LOC:(i + 1) * E_LOC],
        }
        for i in range(N_CORES)
    ]

    LAST_RESULT = run_bass_kernel_spmd(_NC, in_maps, list(range(N_CORES)))
    out_t = np.concatenate(
        [np.asarray(LAST_RESULT.results[i]["out_t"]).astype(np.float32)
         for i in range(N_CORES)], axis=0)

    yf = out_t[e_idx, :, np.minimum(pos, C - 1)]
    yf[~kept] = 0.0
    return np.ascontiguousarray(yf.reshape(B, T, D), dtype=np.float32)


# revision 10
# speedup vs baseline: 1.0402x; 1.0402x over previous
"""CMoE hash-routed expert FFN on 8 NeuronCores (expert-parallel).

Host side (the shard/unshard steps): compute hash routing
e = (token_id % 5099) % 64, first-come slot assignment with capacity 512,
scatter tokens into a per-expert [E, D, C] buffer (transposed, bf16), and
shard 8 experts to each of the 8 cores along with that core's (transposed,
bf16) expert weights.  Device side: per expert
    h  = relu(A @ Wk^T)^2        [C, F]
    kv = h @ Wv^T                [C, D]
    r  = sigmoid(A @ Wr^T)       [C, D]
    out = r * kv
computed entirely in transposed form (contraction dim on SBUF partitions),
bf16 matmul operands with fp32 PSUM accumulation.  Host gathers each
token's slot back out of [E, D, C] and zeroes dropped tokens.

DMA discipline: transfers are issued in strict first-need order, chunked so
compute gates on partial tensors (wk in 4 f-chunks, wv in 2 fo-chunks), and
spread over three rings (sync HWDGE: at+wv, scalar HWDGE: wk, gpsimd SWDGE:
wr + output stores).  The r phase of expert e+1 is emitted after kv(e); its
at/wr loads are issued two experts ahead.  Outputs are stored as bf16 to
halve store traffic; the host upcasts.
"""

import numpy as np
import ml_dtypes

import concourse.bass as bass
import concourse.mybir as mybir
import concourse.tile as tile
from concourse import bacc
from concourse.bass import ts
from concourse.bass_utils import run_bass_kernel_spmd

HASH_PRIME = 5099
B, T, D, F, E = 8, 4096, 512, 1792, 64
S = B * T
C = 512  # capacity = max(4, ceil(S/E))
N_CORES = 8
E_LOC = E // N_CORES  # experts per core

BF16 = mybir.dt.bfloat16
F32 = mybir.dt.float32

_NC = None  # cached compiled Bass program
LAST_RESULT = None  # BassKernelResults of the most recent run (for test.py)


def _build_nc(e_loc=E_LOC, d=D, f=F, c=C):
    """One SPMD program: each core computes e_loc experts' FFN."""
    kd = d // 128   # contraction tiles over D
    kf = f // 128   # contraction tiles over F
    nc = bacc.Bacc("TRN2", target_bir_lowering=False, debug=False,
                   num_devices=N_CORES)

    # at and wr are packed host-side into one [2, d, c] block per expert so
    # the head-critical load (r needs both) is a single full-rate transfer
    awr_t = nc.dram_tensor("awr_t", [e_loc, 2, d, c], BF16, kind="ExternalInput")
    wk_t = nc.dram_tensor("wk_t", [e_loc, d, f], BF16, kind="ExternalInput")
    wv_t = nc.dram_tensor("wv_t", [e_loc, f, d], BF16, kind="ExternalInput")
    out_t = nc.dram_tensor("out_t", [e_loc, d, c], BF16, kind="ExternalOutput")

    with tile.TileContext(nc) as tc:
        with (
            tc.tile_pool(name="wts", bufs=2) as wts,
            tc.tile_pool(name="acts", bufs=2) as acts,
            tc.tile_pool(name="ph", bufs=3, space="PSUM") as ph,
            tc.tile_pool(name="pr", bufs=3, space="PSUM") as pr,
            tc.tile_pool(name="pkv", bufs=2, space="PSUM") as pkv,
        ):
            tiles_awr = {}
            tiles_wk = {}
            tiles_wv = {}
            sigs = {}
            hbs = {}

            # Warm the PE (HAM throttles it to 1.2 GHz until ~3.4us of
            # sustained work) with matmuls on scratch data while the first
            # input DMAs ramp up; the result is never read.  7 cold MMs
            # bridge ~3us, landing right when awr0 arrives.
            warm_l = wts.tile([128, 128], BF16, tag="warm_l")
            warm_r = wts.tile([128, c], BF16, tag="warm_r")
            nc.any.memset(warm_l[:], 0.0)
            nc.any.memset(warm_r[:], 0.0)
            for _ in range(7):
                warm_p = pr.tile([128, c], F32, tag="psr")
                nc.tensor.matmul(warm_p[:], lhsT=warm_l[:], rhs=warm_r[:],
                                 start=True, stop=True)

            def load_at_wr(e):
                # one 1MB transfer; SDMA round-robins across queues, so a
                # single transfer on its own ring completes ~2x faster than
                # two competing halves on separate rings
                awr = wts.tile([128, 2 * kd, c], BF16, tag="awr")
                tiles_awr[e] = awr
                nc.sync.dma_start(
                    awr[:],
                    awr_t[e].rearrange("two (ko p) c -> p (two ko) c", p=128))

            def load_wk(e):
                wk = wts.tile([128, kd, f], BF16, tag="wk")
                tiles_wk[e] = wk
                src = wk_t[e].rearrange("(ko p) f -> p ko f", p=128)
                # 4 chunks: h ft-groups gate on the 448-col chunk they read
                for lo in range(0, f, 512):
                    hi = min(lo + 512, f)
                    nc.sync.dma_start(wk[:, :, lo:hi], src[:, :, lo:hi])

            def load_wv(e):
                wv = wts.tile([128, kf, d], BF16, tag="wv")
                tiles_wv[e] = wv
                src = wv_t[e].rearrange("(fo p) g -> p fo g", p=128)
                half = kf // 2
                nc.sync.dma_start(wv[:, :half, :], src[:, :half, :])
                nc.sync.dma_start(wv[:, half:, :], src[:, half:, :])

            def emit_r(e):
                awr = tiles_awr[e]  # [:, :kd] = A^T rows, [:, kd:] = Wr^T rows
                sig = acts.tile([128, kd, c], F32, tag="sig")
                sigs[e] = sig
                for gt in range(kd):
                    psum_r = pr.tile([128, c], F32, tag="psr")
                    for kt in range(kd):
                        nc.tensor.matmul(
                            psum_r[:],
                            lhsT=awr[:, kd + kt, ts(gt, 128)],
                            rhs=awr[:, kt, :],
                            start=(kt == 0),
                            stop=(kt == kd - 1),
                        )
                    nc.scalar.activation(sig[:, gt, :], psum_r[:],
                                         mybir.ActivationFunctionType.Sigmoid)

            def emit_h(e):
                awr = tiles_awr[e]
                wk = tiles_wk.pop(e)
                # h^T[f, c] = (relu(Wk^T.T @ A^T))^2, bf16 for matmul 2
                hb = acts.tile([128, kf, c], BF16, tag="hb")
                hbs[e] = hb
                for ft in range(kf):
                    psum_h = ph.tile([128, c], F32, tag="psh")
                    for kt in range(kd):
                        nc.tensor.matmul(
                            psum_h[:],
                            lhsT=wk[:, kt, ts(ft, 128)],
                            rhs=awr[:, kt, :],
                            start=(kt == 0),
                            stop=(kt == kd - 1),
                        )
                    nc.scalar.activation(hb[:, ft, :], psum_h[:],
                                         mybir.ActivationFunctionType.Relu)
                    nc.vector.tensor_mul(hb[:, ft, :], hb[:, ft, :], hb[:, ft, :])

            def emit_kv(e):
                tiles_awr.pop(e)
                wv = tiles_wv.pop(e)
                hb = hbs.pop(e)
                sig = sigs.pop(e)
                # kv^T[dd, c] = Wv^T.T @ h^T ; out = sig * kv
                ob = acts.tile([128, kd, c], BF16, tag="ob")
                for dt in range(kd):
                    psum_kv = pkv.tile([128, c], F32, tag="pskv")
                    for ft in range(kf):
                        nc.tensor.matmul(
                            psum_kv[:],
                            lhsT=wv[:, ft, ts(dt, 128)],
                            rhs=hb[:, ft, :],
                            start=(ft == 0),
                            stop=(ft == kf - 1),
                        )
                    nc.vector.tensor_mul(ob[:, dt, :], psum_kv[:], sig[:, dt, :])
                    # store each d-tile as it finishes; the last expert's
                    # stores ride the by-then-idle sync HWDGE ring (lower
                    # latency than SWDGE) to shorten the kernel tail
                    dst = out_t[e].rearrange("(ko p) c -> p ko c", p=128)[:, dt, :]
                    if e == e_loc - 1:
                        nc.sync.dma_start(dst, ob[:, dt, :])
                    else:
                        nc.gpsimd.dma_start(dst, ob[:, dt, :])

            # ALL loads ride the single sync HWDGE ring in strict
            # first-need order: the 16 SDMA engines round-robin across
            # QUEUES, so a lone queue gets full bandwidth and FIFO order
            # within it is exactly the priority order we want.  Stores go
            # on the gpsimd SWDGE ring; the scalar engine runs only
            # activations (a DMA trigger blocking on a full ring there
            # would stall relu/sigmoid and starve the PE).
            load_at_wr(0)
            load_wk(0)
            emit_r(0)
            load_wv(0)
            if e_loc > 1:
                load_at_wr(1)
                load_wk(1)
            for e in range(e_loc):
                emit_h(e)
                emit_kv(e)
                if e + 1 < e_loc:
                    emit_r(e + 1)
                    load_wv(e + 1)
                if e + 2 < e_loc:
                    load_at_wr(e + 2)
                    load_wk(e + 2)

    nc.compile()
    return nc


def _route(token_ids):
    tid = token_ids.reshape(S).astype(np.int64)
    e_idx = (tid % HASH_PRIME) % E
    order = np.argsort(e_idx, kind="stable")
    sorted_e = e_idx[order]
    starts = np.searchsorted(sorted_e, np.arange(E))
    pos = np.empty(S, np.int64)
    pos[order] = np.arange(S) - starts[sorted_e]
    kept = pos < C
    return e_idx, pos, kept


def kernel(x, token_ids, Wk, Wr, Wv):
    global _NC, LAST_RESULT
    if _NC is None:
        _NC = _build_nc()

    e_idx, pos, kept = _route(token_ids)

    bf16 = ml_dtypes.bfloat16
    xf = np.ascontiguousarray(x, dtype=np.float32).reshape(S, D)
    # awr_t[e] = [A^T (dispatched) ; Wr^T] packed so the device loads both
    # with one transfer
    awr_t = np.zeros((E, 2, D, C), bf16)
    disp_t = np.zeros((E, D, C), np.float32)
    disp_t[e_idx[kept], :, pos[kept]] = xf[kept]
    awr_t[:, 0] = disp_t.astype(bf16)
    awr_t[:, 1] = np.asarray(Wr, dtype=np.float32).transpose(0, 2, 1).astype(bf16)

    wk_t = np.asarray(Wk, dtype=np.float32).transpose(0, 2, 1).astype(bf16)
    wv_t = np.asarray(Wv, dtype=np.float32).transpose(0, 2, 1).astype(bf16)

    in_maps = [
        {
            "awr_t": awr_t[i * E_LOC:(i + 1) * E_LOC],
            "wk_t": wk_t[i * E_LOC:(i + 1) * E_LOC],
            "wv_t": wv_t[i * E_LOC:(i + 1) * E_LOC],
        }
        for i in range(N_CORES)
    ]

    LAST_RESULT = run_bass_kernel_spmd(_NC, in_maps, list(range(N_CORES)))
    out_t = np.concatenate(
        [np.asarray(LAST_RESULT.results[i]["out_t"]).astype(np.float32)
         for i in range(N_CORES)], axis=0)

    yf = out_t[e_idx, :, np.minimum(pos, C - 1)]
    yf[~kept] = 0.0
    return np.ascontiguousarray(yf.reshape(B, T, D), dtype=np.float32)


# revision 11
# speedup vs baseline: 1.0517x; 1.0111x over previous
"""CMoE hash-routed expert FFN on 8 NeuronCores (expert-parallel).

Host side (the shard/unshard steps): compute hash routing
e = (token_id % 5099) % 64, first-come slot assignment with capacity 512,
scatter tokens into a per-expert [E, D, C] buffer (transposed, bf16).
Experts are rank-sorted by their effective token count and dealt across the
8 cores so position p on every core has a similar count; the SPMD program
is compiled with a per-position column count N_p = max over cores (rounded
up to 8), skipping compute on empty capacity slots.  Device side: per
expert
    h  = relu(A @ Wk^T)^2        [C, F]
    kv = h @ Wv^T                [C, D]
    r  = sigmoid(A @ Wr^T)       [C, D]
    out = r * kv
computed entirely in transposed form (contraction dim on SBUF partitions),
bf16 matmul operands with fp32 PSUM accumulation.  Host gathers each
token's slot back out of [E, D, C] and zeroes dropped tokens.

DMA discipline: ALL loads ride the single sync HWDGE ring in strict
first-need order — the 16 SDMA engines round-robin across QUEUES, so a
lone queue gets full bandwidth and FIFO order within it is exactly the
priority order we want.  wk is chunked in 4 f-chunks and wv in 2
fo-chunks so compute gates on partial tensors.  Stores go on the gpsimd
SWDGE ring; the scalar engine runs only activations (a DMA trigger
blocking on a full ring there would stall relu/sigmoid and starve the
PE).  Outputs are stored as bf16 to halve store traffic; the host
upcasts.
"""

import numpy as np
import ml_dtypes

import concourse.bass as bass
import concourse.mybir as mybir
import concourse.tile as tile
from concourse import bacc
from concourse.bass import ts
from concourse.bass_utils import run_bass_kernel_spmd

HASH_PRIME = 5099
B, T, D, F, E = 8, 4096, 512, 1792, 64
S = B * T
C = 512  # capacity = max(4, ceil(S/E))
N_CORES = 8
E_LOC = E // N_CORES  # experts per core

BF16 = mybir.dt.bfloat16
F32 = mybir.dt.float32

_NC_CACHE = {}  # pos_ns tuple -> compiled Bass program
LAST_RESULT = None  # BassKernelResults of the most recent run (for test.py)


def _build_nc(pos_ns, d=D, f=F, c=C):
    """One SPMD program: each core computes len(pos_ns) experts' FFN,
    with position p's matmuls using free dim pos_ns[p] <= c."""
    e_loc = len(pos_ns)
    kd = d // 128   # contraction tiles over D
    kf = f // 128   # contraction tiles over F
    nc = bacc.Bacc("TRN2", target_bir_lowering=False, debug=False,
                   num_devices=N_CORES)

    # at and wr are packed host-side into one [2, d, c] block per expert so
    # the head-critical loads are two back-to-back transfers on one ring
    awr_t = nc.dram_tensor("awr_t", [e_loc, 2, d, c], BF16, kind="ExternalInput")
    wk_t = nc.dram_tensor("wk_t", [e_loc, d, f], BF16, kind="ExternalInput")
    wv_t = nc.dram_tensor("wv_t", [e_loc, f, d], BF16, kind="ExternalInput")
    out_t = nc.dram_tensor("out_t", [e_loc, d, c], BF16, kind="ExternalOutput")

    with tile.TileContext(nc) as tc:
        with (
            tc.tile_pool(name="wts", bufs=2) as wts,
            tc.tile_pool(name="acts", bufs=2) as acts,
            tc.tile_pool(name="ph", bufs=3, space="PSUM") as ph,
            tc.tile_pool(name="pr", bufs=3, space="PSUM") as pr,
            tc.tile_pool(name="pkv", bufs=2, space="PSUM") as pkv,
        ):
            tiles_awr = {}
            tiles_wk = {}
            tiles_wv = {}
            sigs = {}
            hbs = {}

            # Warm the PE (HAM throttles it to 1.2 GHz until ~3.4us of
            # sustained work) with matmuls on scratch data while the first
            # input DMAs ramp up; the result is never read.  8 cold MMs
            # bridge ~3.4us, landing right when at0/wr0 arrive.
            warm_l = wts.tile([128, 128], BF16, tag="warm_l")
            warm_r = wts.tile([128, c], BF16, tag="warm_r")
            nc.any.memset(warm_l[:], 0.0)
            nc.any.memset(warm_r[:], 0.0)
            for _ in range(8):
                warm_p = pr.tile([128, c], F32, tag="psr")
                nc.tensor.matmul(warm_p[:], lhsT=warm_l[:], rhs=warm_r[:],
                                 start=True, stop=True)

            def load_at_wr(e):
                n = pos_ns[e]
                awr = wts.tile([128, 2 * kd, c], BF16, tag="awr")
                tiles_awr[e] = awr
                src = awr_t[e].rearrange("two (ko p) c -> p two ko c", p=128)
                # at first, wr second: r(e) is gated by the later arrival,
                # and two transfers pipeline their fixed costs in the FIFO
                nc.sync.dma_start(awr[:, :kd, :n], src[:, 0, :, :n])
                nc.sync.dma_start(awr[:, kd:, :], src[:, 1, :, :])

            def load_wk(e):
                wk = wts.tile([128, kd, f], BF16, tag="wk")
                tiles_wk[e] = wk
                src = wk_t[e].rearrange("(ko p) f -> p ko f", p=128)
                # 4 chunks: h ft-groups gate on the 512-col chunk they read
                for lo in range(0, f, 512):
                    hi = min(lo + 512, f)
                    nc.sync.dma_start(wk[:, :, lo:hi], src[:, :, lo:hi])

            def load_wv(e):
                wv = wts.tile([128, kf, d], BF16, tag="wv")
                tiles_wv[e] = wv
                src = wv_t[e].rearrange("(fo p) g -> p fo g", p=128)
                half = kf // 2
                nc.sync.dma_start(wv[:, :half, :], src[:, :half, :])
                nc.sync.dma_start(wv[:, half:, :], src[:, half:, :])

            def emit_r(e):
                n = pos_ns[e]
                awr = tiles_awr[e]  # [:, :kd] = A^T rows, [:, kd:] = Wr^T rows
                sig = acts.tile([128, kd, c], F32, tag="sig")
                sigs[e] = sig
                for gt in range(kd):
                    psum_r = pr.tile([128, c], F32, tag="psr")
                    for kt in range(kd):
                        nc.tensor.matmul(
                            psum_r[:, :n],
                            lhsT=awr[:, kd + kt, ts(gt, 128)],
                            rhs=awr[:, kt, :n],
                            start=(kt == 0),
                            stop=(kt == kd - 1),
                        )
                    nc.scalar.activation(sig[:, gt, :n], psum_r[:, :n],
                                         mybir.ActivationFunctionType.Sigmoid)

            def emit_h(e):
                n = pos_ns[e]
                awr = tiles_awr[e]
                wk = tiles_wk.pop(e)
                # h^T[f, c] = (relu(Wk^T.T @ A^T))^2, bf16 for matmul 2
                hb = acts.tile([128, kf, c], BF16, tag="hb")
                hbs[e] = hb
                for ft in range(kf):
                    psum_h = ph.tile([128, c], F32, tag="psh")
                    for kt in range(kd):
                        nc.tensor.matmul(
                            psum_h[:, :n],
                            lhsT=wk[:, kt, ts(ft, 128)],
                            rhs=awr[:, kt, :n],
                            start=(kt == 0),
                            stop=(kt == kd - 1),
                        )
                    nc.scalar.activation(hb[:, ft, :n], psum_h[:, :n],
                                         mybir.ActivationFunctionType.Relu)
                    nc.vector.tensor_mul(hb[:, ft, :n], hb[:, ft, :n],
                                         hb[:, ft, :n])

            def emit_kv(e):
                n = pos_ns[e]
                tiles_awr.pop(e)
                wv = tiles_wv.pop(e)
                hb = hbs.pop(e)
                sig = sigs.pop(e)
                # kv^T[dd, c] = Wv^T.T @ h^T ; out = sig * kv
                ob = acts.tile([128, kd, c], BF16, tag="ob")
                dst_all = out_t[e].rearrange("(ko p) c -> p ko c", p=128)
                for dt in range(kd):
                    psum_kv = pkv.tile([128, c], F32, tag="pskv")
                    for ft in range(kf):
                        nc.tensor.matmul(
                            psum_kv[:, :n],
                            lhsT=wv[:, ft, ts(dt, 128)],
                            rhs=hb[:, ft, :n],
                            start=(ft == 0),
                            stop=(ft == kf - 1),
                        )
                    last = (e == e_loc - 1)
                    if last and dt == kd - 1:
                        # split the final mul+store so the kernel tail only
                        # waits on a half-size transfer
                        h1 = n // 2
                        for lo, hi in ((0, h1), (h1, n)):
                            nc.vector.tensor_mul(ob[:, dt, lo:hi],
                                                 psum_kv[:, lo:hi],
                                                 sig[:, dt, lo:hi])
                            nc.sync.dma_start(dst_all[:, dt, lo:hi],
                                              ob[:, dt, lo:hi])
                    else:
                        nc.vector.tensor_mul(ob[:, dt, :n], psum_kv[:, :n],
                                             sig[:, dt, :n])
                        # store each d-tile as it finishes; the last
                        # expert's stores ride the by-then-idle sync HWDGE
                        # ring (lower latency than SWDGE)
                        dst = dst_all[:, dt, :n]
                        if last:
                            nc.sync.dma_start(dst, ob[:, dt, :n])
                        else:
                            nc.gpsimd.dma_start(dst, ob[:, dt, :n])

            load_at_wr(0)
            load_wk(0)
            emit_r(0)
            load_wv(0)
            if e_loc > 1:
                load_at_wr(1)
                load_wk(1)
            for e in range(e_loc):
                emit_h(e)
                emit_kv(e)
                if e + 1 < e_loc:
                    emit_r(e + 1)
                    load_wv(e + 1)
                if e + 2 < e_loc:
                    load_at_wr(e + 2)
                    load_wk(e + 2)

    nc.compile()
    return nc


def _route(token_ids):
    tid = token_ids.reshape(S).astype(np.int64)
    e_idx = (tid % HASH_PRIME) % E
    order = np.argsort(e_idx, kind="stable")
    sorted_e = e_idx[order]
    starts = np.searchsorted(sorted_e, np.arange(E))
    pos = np.empty(S, np.int64)
    pos[order] = np.arange(S) - starts[sorted_e]
    kept = pos < C
    return e_idx, pos, kept


def kernel(x, token_ids, Wk, Wr, Wv):
    global LAST_RESULT

    e_idx, pos, kept = _route(token_ids)

    # Effective token count per expert; rank-sort and deal across cores so
    # each position p has similar counts on all 8 cores.
    counts = np.bincount(e_idx, minlength=E)
    eff = np.minimum(counts, C)
    order = np.argsort(-eff, kind="stable")   # expert ids, biggest first
    # expert order[8p + i] -> core i, position p
    grid = order.reshape(E_LOC, N_CORES)      # [position, core]
    pos_ns = tuple(int(min(C, -8 * (-int(eff[grid[p]].max()) // 8)))
                   for p in range(E_LOC))

    nc = _NC_CACHE.get(pos_ns)
    if nc is None:
        nc = _NC_CACHE[pos_ns] = _build_nc(pos_ns)

    bf16 = ml_dtypes.bfloat16
    xf = np.ascontiguousarray(x, dtype=np.float32).reshape(S, D)
    disp_t = np.zeros((E, D, C), np.float32)
    disp_t[e_idx[kept], :, pos[kept]] = xf[kept]
    awr_t = np.zeros((E, 2, D, C), bf16)
    awr_t[:, 0] = disp_t.astype(bf16)
    awr_t[:, 1] = np.asarray(Wr, dtype=np.float32).transpose(0, 2, 1).astype(bf16)

    wk_t = np.asarray(Wk, dtype=np.float32).transpose(0, 2, 1).astype(bf16)
    wv_t = np.asarray(Wv, dtype=np.float32).transpose(0, 2, 1).astype(bf16)

    in_maps = [
        {
            "awr_t": awr_t[grid[:, i]],
            "wk_t": wk_t[grid[:, i]],
            "wv_t": wv_t[grid[:, i]],
        }
        for i in range(N_CORES)
    ]

    LAST_RESULT = run_bass_kernel_spmd(nc, in_maps, list(range(N_CORES)))
    out_t = np.zeros((E, D, C), np.float32)
    for i in range(N_CORES):
        res = np.asarray(LAST_RESULT.results[i]["out_t"]).astype(np.float32)
        for p in range(E_LOC):
            n = pos_ns[p]
            out_t[grid[p, i], :, :n] = res[p, :, :n]

    yf = out_t[e_idx, :, np.minimum(pos, C - 1)]
    yf[~kept] = 0.0
    return np.ascontiguousarray(yf.reshape(B, T, D), dtype=np.float32)


# revision 14
# speedup vs baseline: 1.1211x; 1.0659x over previous
"""CMoE hash-routed expert FFN on 8 NeuronCores (expert-parallel).

Host side (the shard/unshard steps): compute hash routing
e = (token_id % 5099) % 64, first-come slot assignment with capacity 512,
scatter tokens into a per-expert [E, D, C] buffer (transposed, bf16).
Experts are rank-sorted by their effective token count and dealt across the
8 cores so position p on every core has a similar count; the SPMD program
is compiled with a per-position column count N_p = max over cores (rounded
up to 16), skipping compute on empty capacity slots.  Device side: per
expert
    h  = relu(A @ Wk^T)^2        [C, F]   bf16 matmuls
    kv = h @ Wv^T                [C, D]   bf16 matmuls
    r  = sigmoid(A @ Wr^T)       [C, D]   fp8e4 DoubleRow matmuls (2x rate)
    out = r * kv
computed entirely in transposed form (contraction dim on SBUF partitions),
fp32 PSUM accumulation.  The r path tolerates fp8: sigmoid'(z) <= 1/4
damps the quantization error, and A/Wr are pre-scaled by 16/1024 so their
values sit in e4m3's normal range (the sigmoid activation descales by
2^-14).  DoubleRow packs 2 fp8 weights per PE cell: contraction 512 runs
as 2 MMs of (128 partitions x 2) instead of 4 of 128.  Host gathers each
token's slot back out of [E, D, C] and zeroes dropped tokens.

DMA discipline: ALL loads ride the single sync HWDGE ring in strict
first-need order — the 16 SDMA engines round-robin across QUEUES, so a
lone queue gets full bandwidth and FIFO order within it is exactly the
priority order we want.  wk is chunked in 4 f-chunks and wv in 2
fo-chunks so compute gates on partial tensors.  Stores go on the gpsimd
SWDGE ring; the scalar engine runs only activations (a DMA trigger
blocking on a full ring there would stall relu/sigmoid and starve the
PE).  Outputs are stored as bf16 to halve store traffic; the host
upcasts.
"""

import numpy as np
import ml_dtypes

import concourse.bass as bass
import concourse.mybir as mybir
import concourse.tile as tile
from concourse import bacc
from concourse.bass import ts
from concourse.bass_utils import run_bass_kernel_spmd

HASH_PRIME = 5099
B, T, D, F, E = 8, 4096, 512, 1792, 64
S = B * T
C = 512  # capacity = max(4, ceil(S/E))
N_CORES = 8
E_LOC = E // N_CORES  # experts per core

BF16 = mybir.dt.bfloat16
FP8 = mybir.dt.float8e4
F32 = mybir.dt.float32

A_SCALE = 16.0      # x ~ N(0,1): 16x keeps |x| in e4m3 normal range, max ~90
WR_SCALE = 1024.0   # Wr ~ 0.02*N(0,1): 1024x -> max ~100 < 240
R_DESCALE = 1.0 / (A_SCALE * WR_SCALE)

_NC_CACHE = {}  # pos_ns tuple -> compiled Bass program
LAST_RESULT = None  # BassKernelResults of the most recent run (for test.py)


def _build_nc(pos_ns, d=D, f=F, c=C):
    """One SPMD program: each core computes len(pos_ns) experts' FFN,
    with position p's matmuls using free dim pos_ns[p] <= c."""
    e_loc = len(pos_ns)
    kd = d // 128   # contraction tiles over D
    kf = f // 128   # contraction tiles over F
    nc = bacc.Bacc("TRN2", target_bir_lowering=False, debug=False,
                   num_devices=N_CORES)

    a_t = nc.dram_tensor("a_t", [e_loc, d, c], BF16, kind="ExternalInput")
    # fp8 copies for the r path, packed [A^T * 16 ; Wr^T * 1024]
    q8_t = nc.dram_tensor("q8_t", [e_loc, 2, d, c], FP8, kind="ExternalInput")
    wk_t = nc.dram_tensor("wk_t", [e_loc, d, f], BF16, kind="ExternalInput")
    wv_t = nc.dram_tensor("wv_t", [e_loc, f, d], BF16, kind="ExternalInput")
    out_t = nc.dram_tensor("out_t", [e_loc, d, c], BF16, kind="ExternalOutput")

    with tile.TileContext(nc) as tc:
        with (
            tc.tile_pool(name="sb", bufs=2) as sb,
            tc.tile_pool(name="ph", bufs=3, space="PSUM") as ph,
            tc.tile_pool(name="pr", bufs=3, space="PSUM") as pr,
            tc.tile_pool(name="pkv", bufs=2, space="PSUM") as pkv,
        ):
            tiles_q8 = {}
            tiles_at = {}
            tiles_wk = {}
            tiles_wv = {}
            sigs = {}
            hbs = {}

            # Warm the PE (HAM throttles it to 1.2 GHz until ~3.4us of
            # sustained work) with matmuls on scratch data while the first
            # input DMAs ramp up; the result is never read.  8 cold MMs
            # bridge ~3.4us, landing right when q8(0) arrives.
            warm_l = sb.tile([128, 128], BF16, tag="warm_l")
            warm_r = sb.tile([128, c], BF16, tag="warm_r")
            nc.any.memset(warm_l[:], 0.0)
            nc.any.memset(warm_r[:], 0.0)
            for _ in range(8):
                warm_p = pr.tile([128, c], F32, tag="psr")
                nc.tensor.matmul(warm_p[:], lhsT=warm_l[:], rhs=warm_r[:],
                                 start=True, stop=True)

            def load_q8(e):
                q8 = sb.tile([128, 2 * kd, c], FP8, tag="q8")
                tiles_q8[e] = q8
                nc.sync.dma_start(
                    q8[:], q8_t[e].rearrange("two (ko p) c -> p (two ko) c",
                                             p=128))

            def load_at(e):
                n = pos_ns[e]
                at = sb.tile([128, kd, c], BF16, tag="at")
                tiles_at[e] = at
                src = a_t[e].rearrange("(ko p) c -> p ko c", p=128)
                nc.sync.dma_start(at[:, :, :n], src[:, :, :n])

            def load_wk(e):
                wk = sb.tile([128, kd, f], BF16, tag="wk")
                tiles_wk[e] = wk
                src = wk_t[e].rearrange("(ko p) f -> p ko f", p=128)
                # 4 chunks: h ft-groups gate on the 512-col chunk they read
                for lo in range(0, f, 512):
                    hi = min(lo + 512, f)
                    nc.sync.dma_start(wk[:, :, lo:hi], src[:, :, lo:hi])

            def load_wv(e):
                wv = sb.tile([128, kf, d], BF16, tag="wv")
                tiles_wv[e] = wv
                src = wv_t[e].rearrange("(fo p) g -> p fo g", p=128)
                half = kf // 2
                nc.sync.dma_start(wv[:, :half, :], src[:, :half, :])
                nc.sync.dma_start(wv[:, half:, :], src[:, half:, :])

            def emit_r(e):
                n = pos_ns[e]
                q8 = tiles_q8.pop(e)  # [:, :kd] = 16*A^T, [:, kd:] = 1024*Wr^T
                sig = sb.tile([128, kd, c], F32, tag="sig")
                sigs[e] = sig
                for gt in range(kd):
                    psum_r = pr.tile([128, c], F32, tag="psr")
                    for kt2 in range(2):
                        nc.tensor.matmul(
                            psum_r[:, :n],
                            lhsT=q8[:, kd + 2 * kt2:kd + 2 * kt2 + 2,
                                    ts(gt, 128)],
                            rhs=q8[:, 2 * kt2:2 * kt2 + 2, :n],
                            start=(kt2 == 0),
                            stop=(kt2 == 1),
                            perf_mode=mybir.MatmulPerfMode.DoubleRow,
                        )
                    nc.scalar.activation(sig[:, gt, :n], psum_r[:, :n],
                                         mybir.ActivationFunctionType.Sigmoid,
                                         scale=R_DESCALE)

            def emit_h(e):
                n = pos_ns[e]
                at = tiles_at.pop(e)
                wk = tiles_wk.pop(e)
                # h^T[f, c] = (relu(Wk^T.T @ A^T))^2, bf16 for matmul 2
                hb = sb.tile([128, kf, c], BF16, tag="hb")
                hbs[e] = hb
                for ft in range(kf):
                    psum_h = ph.tile([128, c], F32, tag="psh")
                    for kt in range(kd):
                        nc.tensor.matmul(
                            psum_h[:, :n],
                            lhsT=wk[:, kt, ts(ft, 128)],
                            rhs=at[:, kt, :n],
                            start=(kt == 0),
                            stop=(kt == kd - 1),
                        )
                    nc.scalar.activation(hb[:, ft, :n], psum_h[:, :n],
                                         mybir.ActivationFunctionType.Relu)
                    nc.vector.tensor_mul(hb[:, ft, :n], hb[:, ft, :n],
                                         hb[:, ft, :n])

            def emit_kv(e):
                n = pos_ns[e]
                wv = tiles_wv.pop(e)
                hb = hbs.pop(e)
                sig = sigs.pop(e)
                # kv^T[dd, c] = Wv^T.T @ h^T ; out = sig * kv
                ob = sb.tile([128, kd, c], BF16, tag="ob")
                dst_all = out_t[e].rearrange("(ko p) c -> p ko c", p=128)
                for dt in range(kd):
                    psum_kv = pkv.tile([128, c], F32, tag="pskv")
                    for ft in range(kf):
                        nc.tensor.matmul(
                            psum_kv[:, :n],
                            lhsT=wv[:, ft, ts(dt, 128)],
                            rhs=hb[:, ft, :n],
                            start=(ft == 0),
                            stop=(ft == kf - 1),
                        )
                    last = (e == e_loc - 1)
                    if last and dt == kd - 1:
                        # split the final mul+store so the kernel tail only
                        # waits on a half-size transfer
                        h1 = n // 2
                        for lo, hi in ((0, h1), (h1, n)):
                            nc.vector.tensor_mul(ob[:, dt, lo:hi],
                                                 psum_kv[:, lo:hi],
                                                 sig[:, dt, lo:hi])
                            nc.sync.dma_start(dst_all[:, dt, lo:hi],
                                              ob[:, dt, lo:hi])
                    else:
                        nc.vector.tensor_mul(ob[:, dt, :n], psum_kv[:, :n],
                                             sig[:, dt, :n])
                        # store each d-tile as it finishes; the last
                        # expert's stores ride the by-then-idle sync HWDGE
                        # ring (lower latency than SWDGE)
                        dst = dst_all[:, dt, :n]
                        if last:
                            nc.sync.dma_start(dst, ob[:, dt, :n])
                        else:
                            nc.gpsimd.dma_start(dst, ob[:, dt, :n])

            load_q8(0)
            load_at(0)
            load_wk(0)
            emit_r(0)
            load_wv(0)
            if e_loc > 1:
                load_q8(1)
                load_at(1)
                load_wk(1)
            for e in range(e_loc):
                emit_h(e)
                emit_kv(e)
                if e + 1 < e_loc:
                    emit_r(e + 1)
                    load_wv(e + 1)
                if e + 2 < e_loc:
                    load_q8(e + 2)
                    load_at(e + 2)
                    load_wk(e + 2)

    nc.compile()
    return nc


def _route(token_ids):
    tid = token_ids.reshape(S).astype(np.int64)
    e_idx = (tid % HASH_PRIME) % E
    order = np.argsort(e_idx, kind="stable")
    sorted_e = e_idx[order]
    starts = np.searchsorted(sorted_e, np.arange(E))
    pos = np.empty(S, np.int64)
    pos[order] = np.arange(S) - starts[sorted_e]
    kept = pos < C
    return e_idx, pos, kept


def kernel(x, token_ids, Wk, Wr, Wv):
    global LAST_RESULT

    e_idx, pos, kept = _route(token_ids)

    # Effective token count per expert; rank-sort and deal across cores so
    # each position p has similar counts on all 8 cores.
    counts = np.bincount(e_idx, minlength=E)
    eff = np.minimum(counts, C)
    order = np.argsort(-eff, kind="stable")   # expert ids, biggest first
    # expert order[8p + i] -> core i, position p
    grid = order.reshape(E_LOC, N_CORES)      # [position, core]
    pos_ns = tuple(int(min(C, -16 * (-int(eff[grid[p]].max()) // 16)))
                   for p in range(E_LOC))

    nc = _NC_CACHE.get(pos_ns)
    if nc is None:
        nc = _NC_CACHE[pos_ns] = _build_nc(pos_ns)

    bf16 = ml_dtypes.bfloat16
    fp8 = ml_dtypes.float8_e4m3
    xf = np.ascontiguousarray(x, dtype=np.float32).reshape(S, D)
    disp_t = np.zeros((E, D, C), np.float32)
    disp_t[e_idx[kept], :, pos[kept]] = xf[kept]
    a_t = disp_t.astype(bf16)

    wr_tf = np.asarray(Wr, dtype=np.float32).transpose(0, 2, 1)
    q8_t = np.empty((E, 2, D, C), fp8)
    q8_t[:, 0] = np.clip(disp_t * A_SCALE, -240, 240).astype(fp8)
    q8_t[:, 1] = np.clip(wr_tf * WR_SCALE, -240, 240).astype(fp8)

    wk_t = np.asarray(Wk, dtype=np.float32).transpose(0, 2, 1).astype(bf16)
    wv_t = np.asarray(Wv, dtype=np.float32).transpose(0, 2, 1).astype(bf16)

    in_maps = [
        {
            "a_t": a_t[grid[:, i]],
            "q8_t": q8_t[grid[:, i]],
            "wk_t": wk_t[grid[:, i]],
            "wv_t": wv_t[grid[:, i]],
        }
        for i in range(N_CORES)
    ]

    LAST_RESULT = run_bass_kernel_spmd(nc, in_maps, list(range(N_CORES)))
    out_t = np.zeros((E, D, C), np.float32)
    for i in range(N_CORES):
        res = np.asarray(LAST_RESULT.results[i]["out_t"]).astype(np.float32)
        for p in range(E_LOC):
            n = pos_ns[p]
            out_t[grid[p, i], :, :n] = res[p, :, :n]

    yf = out_t[e_idx, :, np.minimum(pos, C - 1)]
    yf[~kept] = 0.0
    return np.ascontiguousarray(yf.reshape(B, T, D), dtype=np.float32)


# revision 16
# speedup vs baseline: 1.1232x; 1.0019x over previous
"""CMoE hash-routed expert FFN on 8 NeuronCores (expert-parallel).

Host side (the shard/unshard steps): compute hash routing
e = (token_id % 5099) % 64, first-come slot assignment with capacity 512,
scatter tokens into a per-expert [E, D, C] buffer (transposed, bf16).
Experts are rank-sorted by their effective token count and dealt across the
8 cores so position p on every core has a similar count; the SPMD program
is compiled with a per-position column count N_p = max over cores (rounded
up to 16), skipping compute on empty capacity slots.  Device side: per
expert
    h  = relu(A @ Wk^T)^2        [C, F]   bf16 matmuls
    kv = h @ Wv^T                [C, D]   bf16 matmuls
    r  = sigmoid(A @ Wr^T)       [C, D]   fp8e4 DoubleRow matmuls (2x rate)
    out = r * kv
computed entirely in transposed form (contraction dim on SBUF partitions),
fp32 PSUM accumulation.  The r path tolerates fp8: sigmoid'(z) <= 1/4
damps the quantization error, and A/Wr are pre-scaled by 16/1024 so their
values sit in e4m3's normal range (the sigmoid activation descales by
2^-14).  DoubleRow packs 2 fp8 weights per PE cell: contraction 512 runs
as 2 MMs of (128 partitions x 2) instead of 4 of 128.  Host gathers each
token's slot back out of [E, D, C] and zeroes dropped tokens.

DMA discipline: ALL loads ride the single sync HWDGE ring in strict
first-need order — the 16 SDMA engines round-robin across QUEUES, so a
lone queue gets full bandwidth and FIFO order within it is exactly the
priority order we want.  wk is chunked in 4 f-chunks and wv in 2
fo-chunks so compute gates on partial tensors.  Stores go on the gpsimd
SWDGE ring; the scalar engine runs only activations (a DMA trigger
blocking on a full ring there would stall relu/sigmoid and starve the
PE).  Outputs are stored as bf16 to halve store traffic; the host
upcasts.
"""

import numpy as np
import ml_dtypes

# concourse.bass_utils imports antenv.axon_hooks when BASS_TRACE is set; the
# container's antenv stub lacks that module.  Provide an inert registry so
# tracing degrades gracefully instead of crashing.
try:
    import antenv.axon_hooks  # noqa: F401
except ImportError:
    import sys as _sys
    import types as _types
    _m = _types.ModuleType("antenv.axon_hooks")
    _m._hook = None
    def _set(hook):
        _m._hook = hook
    def _get():
        return _m._hook
    _m.set_axon_ntff_profile_hook = _set
    _m.get_axon_ntff_profile_hook = _get
    _sys.modules["antenv.axon_hooks"] = _m

import concourse.bass as bass
import concourse.mybir as mybir
import concourse.tile as tile
from concourse import bacc
from concourse.bass import ts
from concourse.bass_utils import run_bass_kernel_spmd

HASH_PRIME = 5099
B, T, D, F, E = 8, 4096, 512, 1792, 64
S = B * T
C = 512  # capacity = max(4, ceil(S/E))
N_CORES = 8
E_LOC = E // N_CORES  # experts per core

BF16 = mybir.dt.bfloat16
FP8 = mybir.dt.float8e4
F32 = mybir.dt.float32

A_SCALE = 16.0      # x ~ N(0,1): 16x keeps |x| in e4m3 normal range, max ~90
WR_SCALE = 1024.0   # Wr ~ 0.02*N(0,1): 1024x -> max ~100 < 240
R_DESCALE = 1.0 / (A_SCALE * WR_SCALE)

_NC_CACHE = {}  # pos_ns tuple -> compiled Bass program
LAST_RESULT = None  # BassKernelResults of the most recent run (for test.py)


def _build_nc(pos_ns, d=D, f=F, c=C):
    """One SPMD program: each core computes len(pos_ns) experts' FFN,
    with position p's matmuls using free dim pos_ns[p] <= c."""
    e_loc = len(pos_ns)
    kd = d // 128   # contraction tiles over D
    kf = f // 128   # contraction tiles over F
    nc = bacc.Bacc("TRN2", target_bir_lowering=False, debug=False,
                   num_devices=N_CORES)

    a_t = nc.dram_tensor("a_t", [e_loc, d, c], BF16, kind="ExternalInput")
    # fp8 copies for the r path, packed [A^T * 16 ; Wr^T * 1024]
    q8_t = nc.dram_tensor("q8_t", [e_loc, 2, d, c], FP8, kind="ExternalInput")
    wk_t = nc.dram_tensor("wk_t", [e_loc, d, f], BF16, kind="ExternalInput")
    wv_t = nc.dram_tensor("wv_t", [e_loc, f, d], BF16, kind="ExternalInput")
    out_t = nc.dram_tensor("out_t", [e_loc, d, c], BF16, kind="ExternalOutput")

    with tile.TileContext(nc) as tc:
        with (
            tc.tile_pool(name="sb", bufs=2) as sb,
            tc.tile_pool(name="ph", bufs=3, space="PSUM") as ph,
            tc.tile_pool(name="pr", bufs=3, space="PSUM") as pr,
            tc.tile_pool(name="pkv", bufs=2, space="PSUM") as pkv,
        ):
            tiles_q8 = {}
            tiles_at = {}
            tiles_wk = {}
            tiles_wv = {}
            sigs = {}
            hbs = {}

            # Warm the PE (HAM throttles it to 1.2 GHz until ~3.4us of
            # sustained work) with matmuls on scratch data while the first
            # input DMAs ramp up; the result is never read.  8 cold MMs
            # bridge ~3.4us, landing right when q8(0) arrives.
            warm_l = sb.tile([128, 128], BF16, tag="warm_l")
            warm_r = sb.tile([128, c], BF16, tag="warm_r")
            nc.any.memset(warm_l[:], 0.0)
            nc.any.memset(warm_r[:], 0.0)
            for _ in range(7):
                warm_p = pr.tile([128, c], F32, tag="psr")
                nc.tensor.matmul(warm_p[:], lhsT=warm_l[:], rhs=warm_r[:],
                                 start=True, stop=True)

            def load_q8(e):
                q8 = sb.tile([128, 2 * kd, c], FP8, tag="q8")
                tiles_q8[e] = q8
                src = q8_t[e].rearrange("half (ko p) c -> p half ko c", p=128)
                # two half-transfers: r(e)'s kt2=0 MMs gate on the first
                nc.sync.dma_start(q8[:, :kd, :], src[:, 0, :, :])
                nc.sync.dma_start(q8[:, kd:, :], src[:, 1, :, :])

            def load_at(e):
                n = pos_ns[e]
                at = sb.tile([128, kd, c], BF16, tag="at")
                tiles_at[e] = at
                src = a_t[e].rearrange("(ko p) c -> p ko c", p=128)
                nc.sync.dma_start(at[:, :, :n], src[:, :, :n])

            def load_wk(e):
                wk = sb.tile([128, kd, f], BF16, tag="wk")
                tiles_wk[e] = wk
                src = wk_t[e].rearrange("(ko p) f -> p ko f", p=128)
                # 4 chunks: h ft-groups gate on the 512-col chunk they read
                for lo in range(0, f, 512):
                    hi = min(lo + 512, f)
                    nc.sync.dma_start(wk[:, :, lo:hi], src[:, :, lo:hi])

            def load_wv(e):
                wv = sb.tile([128, kf, d], BF16, tag="wv")
                tiles_wv[e] = wv
                src = wv_t[e].rearrange("(fo p) g -> p fo g", p=128)
                half = kf // 2
                nc.sync.dma_start(wv[:, :half, :], src[:, :half, :])
                nc.sync.dma_start(wv[:, half:, :], src[:, half:, :])

            def emit_r(e):
                n = pos_ns[e]
                # rows [4t..4t+3] = [16*A^T blk 2t, 2t+1; 1024*Wr^T blk 2t, 2t+1]
                q8 = tiles_q8.pop(e)
                sig = sb.tile([128, kd, c], F32, tag="sig")
                sigs[e] = sig
                for gt in range(kd):
                    psum_r = pr.tile([128, c], F32, tag="psr")
                    for kt2 in range(2):
                        nc.tensor.matmul(
                            psum_r[:, :n],
                            lhsT=q8[:, 4 * kt2 + 2:4 * kt2 + 4, ts(gt, 128)],
                            rhs=q8[:, 4 * kt2:4 * kt2 + 2, :n],
                            start=(kt2 == 0),
                            stop=(kt2 == 1),
                            perf_mode=mybir.MatmulPerfMode.DoubleRow,
                        )
                    nc.scalar.activation(sig[:, gt, :n], psum_r[:, :n],
                                         mybir.ActivationFunctionType.Sigmoid,
                                         scale=R_DESCALE)

            def emit_h(e):
                n = pos_ns[e]
                at = tiles_at.pop(e)
                wk = tiles_wk.pop(e)
                # h^T[f, c] = (relu(Wk^T.T @ A^T))^2, bf16 for matmul 2
                hb = sb.tile([128, kf, c], BF16, tag="hb")
                hbs[e] = hb
                for ft in range(kf):
                    psum_h = ph.tile([128, c], F32, tag="psh")
                    for kt in range(kd):
                        nc.tensor.matmul(
                            psum_h[:, :n],
                            lhsT=wk[:, kt, ts(ft, 128)],
                            rhs=at[:, kt, :n],
                            start=(kt == 0),
                            stop=(kt == kd - 1),
                        )
                    nc.scalar.activation(hb[:, ft, :n], psum_h[:, :n],
                                         mybir.ActivationFunctionType.Relu)
                    nc.vector.tensor_mul(hb[:, ft, :n], hb[:, ft, :n],
                                         hb[:, ft, :n])

            def emit_kv(e):
                n = pos_ns[e]
                wv = tiles_wv.pop(e)
                hb = hbs.pop(e)
                sig = sigs.pop(e)
                # kv^T[dd, c] = Wv^T.T @ h^T ; out = sig * kv
                ob = sb.tile([128, kd, c], BF16, tag="ob")
                dst_all = out_t[e].rearrange("(ko p) c -> p ko c", p=128)
                for dt in range(kd):
                    psum_kv = pkv.tile([128, c], F32, tag="pskv")
                    for ft in range(kf):
                        nc.tensor.matmul(
                            psum_kv[:, :n],
                            lhsT=wv[:, ft, ts(dt, 128)],
                            rhs=hb[:, ft, :n],
                            start=(ft == 0),
                            stop=(ft == kf - 1),
                        )
                    if e == e_loc - 1 and dt == kd - 1:
                        # split the final mul+store so the kernel tail only
                        # waits on a half-size transfer
                        h1 = n // 2
                        for lo, hi in ((0, h1), (h1, n)):
                            nc.vector.tensor_mul(ob[:, dt, lo:hi],
                                                 psum_kv[:, lo:hi],
                                                 sig[:, dt, lo:hi])
                            nc.sync.dma_start(dst_all[:, dt, lo:hi],
                                              ob[:, dt, lo:hi])
                    else:
                        nc.vector.tensor_mul(ob[:, dt, :n], psum_kv[:, :n],
                                             sig[:, dt, :n])
                        nc.sync.dma_start(dst_all[:, dt, :n], ob[:, dt, :n])

            load_q8(0)
            load_at(0)
            load_wk(0)
            emit_r(0)
            load_wv(0)
            if e_loc > 1:
                load_q8(1)
                load_at(1)
                load_wk(1)
            for e in range(e_loc):
                emit_h(e)
                emit_kv(e)
                if e + 1 < e_loc:
                    emit_r(e + 1)
                    load_wv(e + 1)
                if e + 2 < e_loc:
                    load_q8(e + 2)
                    load_at(e + 2)
                    load_wk(e + 2)

    nc.compile()
    return nc


def _route(token_ids):
    tid = token_ids.reshape(S).astype(np.int64)
    e_idx = (tid % HASH_PRIME) % E
    order = np.argsort(e_idx, kind="stable")
    sorted_e = e_idx[order]
    starts = np.searchsorted(sorted_e, np.arange(E))
    pos = np.empty(S, np.int64)
    pos[order] = np.arange(S) - starts[sorted_e]
    kept = pos < C
    return e_idx, pos, kept


def kernel(x, token_ids, Wk, Wr, Wv):
    global LAST_RESULT

    e_idx, pos, kept = _route(token_ids)

    # Effective token count per expert; rank-sort and deal across cores so
    # each position p has similar counts on all 8 cores.
    counts = np.bincount(e_idx, minlength=E)
    eff = np.minimum(counts, C)
    order = np.argsort(-eff, kind="stable")   # expert ids, biggest first
    # expert order[8p + i] -> core i, position p
    grid = order.reshape(E_LOC, N_CORES)      # [position, core]
    pos_ns = tuple(int(min(C, -8 * (-int(eff[grid[p]].max()) // 8)))
                   for p in range(E_LOC))

    nc = _NC_CACHE.get(pos_ns)
    if nc is None:
        nc = _NC_CACHE[pos_ns] = _build_nc(pos_ns)

    bf16 = ml_dtypes.bfloat16
    fp8 = ml_dtypes.float8_e4m3
    xf = np.ascontiguousarray(x, dtype=np.float32).reshape(S, D)
    disp_t = np.zeros((E, D, C), np.float32)
    disp_t[e_idx[kept], :, pos[kept]] = xf[kept]
    a_t = disp_t.astype(bf16)

    wr_tf = np.asarray(Wr, dtype=np.float32).transpose(0, 2, 1)
    a8 = np.clip(disp_t * A_SCALE, -240, 240).astype(fp8).reshape(E, 4, 128, C)
    w8 = np.clip(wr_tf * WR_SCALE, -240, 240).astype(fp8).reshape(E, 4, 128, C)
    # half t = [a blk 2t, a blk 2t+1, w blk 2t, w blk 2t+1]
    q8_t = np.empty((E, 2, 4, 128, C), fp8)
    q8_t[:, 0, :2] = a8[:, 0:2]
    q8_t[:, 0, 2:] = w8[:, 0:2]
    q8_t[:, 1, :2] = a8[:, 2:4]
    q8_t[:, 1, 2:] = w8[:, 2:4]
    q8_t = q8_t.reshape(E, 2, D, C)

    wk_t = np.asarray(Wk, dtype=np.float32).transpose(0, 2, 1).astype(bf16)
    wv_t = np.asarray(Wv, dtype=np.float32).transpose(0, 2, 1).astype(bf16)

    in_maps = [
        {
            "a_t": a_t[grid[:, i]],
            "q8_t": q8_t[grid[:, i]],
            "wk_t": wk_t[grid[:, i]],
            "wv_t": wv_t[grid[:, i]],
        }
        for i in range(N_CORES)
    ]

    LAST_RESULT = run_bass_kernel_spmd(nc, in_maps, list(range(N_CORES)))
    out_t = np.zeros((E, D, C), np.float32)
    for i in range(N_CORES):
        res = np.asarray(LAST_RESULT.results[i]["out_t"]).astype(np.float32)
        for p in range(E_LOC):
            n = pos_ns[p]
            out_t[grid[p, i], :, :n] = res[p, :, :n]

    yf = out_t[e_idx, :, np.minimum(pos, C - 1)]
    yf[~kept] = 0.0
    return np.ascontiguousarray(yf.reshape(B, T, D), dtype=np.float32)
